# revision 4
# baseline (speedup 1.0000x reference)
"""LoRA linear kernel for Trainium2 (8 NeuronCores, SPMD data-parallel).

Computes out = x @ (A @ B) for
    x: [4, 2048, 4096] f32, A: [4096, 16] f32, B: [16, 4096] f32
by reassociating to (x @ A) @ B  (2.1 GFLOP instead of 274 GFLOP).

Sharding: x is split row-wise (batch*seq = 8192 rows -> 1024 rows/core),
A and B replicated; no collectives.

Per core, everything runs in bf16 with f32 PSUM accumulation (error
~5e-3 of output scale, well under the 2e-2 gate):

  stage 1:  tT[16, n]   = sum_c A_c[128,16].T @ x_c[128, n]   (PSUM accum)
  stage 2:  out[128, d] = tT[:, rb].T @ B[16, d]              (8x 512-col MMs)

The host pre-tiles x into xP[NCH, 128, KC, RCH] (bf16) so each input DMA
reads 16 KB contiguous per partition (large descriptors -> near-peak HBM
bandwidth; the naive strided layout got 1 KB descriptors and ~24% MBU).
Input DMAs ride the SP HWDGE ring, output DMAs the ACT ring, so they
don't FIFO-block each other. Outputs are written bf16 and upcast on the
host, halving write traffic.
"""

import numpy as np
import ml_dtypes

import concourse.bass as bass
import concourse.bacc as bacc
import concourse.mybir as mybir
from concourse.tile import TileContext
from concourse.bass_utils import run_bass_kernel_spmd

N_CORES = 8
BATCH, SEQ, D_IN, D_OUT, R = 4, 2048, 4096, 4096, 16
ROWS = BATCH * SEQ              # 8192
RPC = ROWS // N_CORES           # 1024 rows per core
KC = D_IN // 128                # 32 contraction chunks of 128
RCH = 256                       # rows per pipeline chunk
NCH = RPC // RCH                # 4 chunks per core
DC = 512                        # d_out columns per stage-2 matmul (PSUM bank)
NDC = D_OUT // DC               # 8

F32 = mybir.dt.float32
BF16 = mybir.dt.bfloat16
NP_BF16 = ml_dtypes.bfloat16

_cache = {}


def _build(out_bf16=True):
    nc = bacc.Bacc("TRN2", target_bir_lowering=False)
    out_dt = BF16 if out_bf16 else F32

    xP = nc.dram_tensor("xP", [NCH, 128, KC, RCH], BF16, kind="ExternalInput")
    Ap = nc.dram_tensor("Ap", [128, KC, R], BF16, kind="ExternalInput")
    Bw = nc.dram_tensor("Bw", [R, D_OUT], BF16, kind="ExternalInput")
    out = nc.dram_tensor("out", [RPC, D_OUT], out_dt, kind="ExternalOutput")

    with TileContext(nc) as tc:
        with (
            tc.tile_pool(name="consts", bufs=1) as cpool,
            tc.tile_pool(name="xin", bufs=3) as xpool,
            tc.tile_pool(name="tbuf", bufs=2) as tpool,
            tc.tile_pool(name="obuf", bufs=3) as opool,
            tc.tile_pool(name="pt", bufs=2, space="PSUM") as ptpool,
            tc.tile_pool(name="po", bufs=4, space="PSUM") as popool,
        ):
            a_tile = cpool.tile([128, KC, R], BF16)
            nc.sync.dma_start(out=a_tile[:], in_=Ap[:, :, :])
            b_tile = cpool.tile([R, D_OUT], BF16)
            nc.sync.dma_start(out=b_tile[:], in_=Bw[:, :])

            for k in range(NCH):
                xt = xpool.tile([128, KC, RCH], BF16)
                nc.sync.dma_start(out=xt[:], in_=xP[k, :, :, :])

                # stage 1: tT [16, RCH] = (x_chunk @ A).T
                pt = ptpool.tile([R, RCH], F32)
                for c in range(KC):
                    nc.tensor.matmul(
                        pt[:],
                        a_tile[:, c, :],
                        xt[:, c, :],
                        start=(c == 0),
                        stop=(c == KC - 1),
                    )
                tT = tpool.tile([R, RCH], BF16)
                nc.vector.tensor_copy(tT[:], pt[:])

                # stage 2: out rows = tT.T @ B, one 128-row block at a time
                for rb in range(RCH // 128):
                    osb = opool.tile([128, D_OUT], out_dt)
                    for dc in range(NDC):
                        po = popool.tile([128, DC], F32)
                        nc.tensor.matmul(
                            po[:],
                            tT[:, rb * 128:(rb + 1) * 128],
                            b_tile[:, dc * DC:(dc + 1) * DC],
                            start=True,
                            stop=True,
                        )
                        dst = osb[:, dc * DC:(dc + 1) * DC]
                        if dc in (2, 5, 7):
                            nc.scalar.copy(out=dst, in_=po[:])
                        else:
                            nc.vector.tensor_copy(dst, po[:])
                    row0 = k * RCH + rb * 128
                    nc.scalar.dma_start(out=out[row0:row0 + 128, :],
                                        in_=osb[:])
    nc.compile()
    return nc


def _get_nc(out_bf16=True):
    key = ("v2", out_bf16)
    if key not in _cache:
        _cache[key] = _build(out_bf16)
    return _cache[key]


def kernel(x, A, B, trace=False, out_bf16=True, **_ignored):
    x = np.asarray(x, dtype=np.float32)
    A = np.asarray(A, dtype=np.float32)
    B = np.asarray(B, dtype=np.float32)
    xf = x.reshape(ROWS, D_IN)

    Ab = np.ascontiguousarray(
        A.astype(NP_BF16).reshape(KC, 128, R).transpose(1, 0, 2))
    Bb = np.ascontiguousarray(B.astype(NP_BF16))

    nc = _get_nc(out_bf16)
    in_maps = []
    for i in range(N_CORES):
        xs = xf[i * RPC:(i + 1) * RPC]                 # [1024, 4096]
        # xP[k, p, c, j] = xs[k*RCH + j, c*128 + p]
        xPc = np.ascontiguousarray(
            xs.astype(NP_BF16).reshape(NCH, RCH, KC, 128).transpose(0, 3, 2, 1))
        in_maps.append({"xP": xPc, "Ap": Ab, "Bw": Bb})

    res = run_bass_kernel_spmd(nc, in_maps, list(range(N_CORES)), trace=trace)
    outs = [res.results[i]["out"] for i in range(N_CORES)]
    full = np.concatenate(outs, axis=0).reshape(BATCH, SEQ, D_OUT)
    full = np.asarray(full, dtype=np.float32)
    if trace:
        kernel.last_exec_time_ns = res.exec_time_ns
        kernel.last_results = res
    return full
